# revision 20
# baseline (speedup 1.0000x reference)
"""Trainium2 Bass kernel for the GNN message-update MLP:

    out = relu(concat([v_i, v_j, e_ij], -1) @ W1 + b1) @ W2 + b2

Strategy (memory-bound, E = 1M edges, data-parallel across 8 cores):
  - Shard edges across the 8 NeuronCores (125000 each, padded to 126976).
  - Everything ships in fp16 (inputs AND outputs): the harness gate is
    rel_err < 2e-2 and fp16 end-to-end measures ~6e-4, so fp32
    split-precision would waste 2x the HBM traffic. Per-core traffic is
    ~57MB -> ~160us at the 358 GB/s HBM/core roofline.
  - One fused input DMA per 4096-edge block: [128, 5120] fp16 where
    cols 0:4096 hold [v_i^T; v_j^T] (128 partitions) and cols 4096:5120
    hold e^T folded from [32, 4096] to [128, 1024] (chunk c of 1024
    edges lives on partitions 32c:32c+32) so the DMA uses all 128
    partitions / all 16 SDMA engines.
  - Matmuls co-execute only when their PE quadrant sets are disjoint
    AND they write disjoint PSUM partition ranges (two concurrent
    streams into the same PSUM region race on has_written - hangs the
    device, measured). So tiles are paired (q, q+4) across the two PE
    column groups and issued as steps of two disjoint-quadrant matmuls:
      step1: xv_A@(0,cg0) , xv_B@(0,cg1)       K=128 each
      step2: e_A@(32c,cg0), e_B@(32c+64,cg1)   K=32, diagonal row grps
             (pairing q with q+4 puts their e-chunks on different row
             halves; e overlaps xv's quadrant so same-PSUM accumulation
             is hardware-serialized - safe)
      VectorE: h = fp16(relu(psum + b1))       one [128,512] op
      step3: L2_A@(0,0) , L2_B@(64,64)         K=64, diagonal
      ScalarE: out = fp16(psum + b2)           one [128,512] op
    ~3 x 512-cycle steps per 1024 edges -> ~80-106us/core PE time,
    under the ~160us DMA floor.
  - Startup ordered to keep the PE HAM clock gate warm: weights fused
    into 2 small DMAs ahead of block-0's input DMA, and the warmup
    matmul run sized to bridge until block-0 data lands. A PE idle gap
    >= ~1us re-throttles the PE clock to 1.2 GHz, usually for the rest
    of the kernel (measured) - hence the warmup, the 7-deep input
    prefetch, and the per-block filler matmuls that keep the PE the
    pacing engine.
  - Output [128, 2048] fp16 per block, unfolded + cast to f32 on host.

Measured: 190.5us HW exec (8 cores), rel err 5.7e-4 vs the fp32
reference (gate 2e-2). Baseline this session started from: 482.9us.
"""

import numpy as np

import concourse.bacc as bacc
import concourse.bass as bass
import concourse.mybir as mybir
import concourse.tile as tile
from concourse.bass_utils import run_bass_kernel_spmd

# ---- problem constants (hardcoded per harness contract) ----
E_TOTAL = 1_000_000
N_CORES = 8
IN_C = 64
IN_E = 32
HID = 64
OUT_C = 64

NHALF = 512                    # edges per matmul (moving free dim, 1 psum bank)
Q_PER_BLK = 8                  # 512-edge tiles per block
BLK_EDGES = NHALF * Q_PER_BLK  # 4096
EPC = E_TOTAL // N_CORES       # 125000 edges per core
N_BLK = -(-EPC // BLK_EDGES)   # 31
EPAD = N_BLK * BLK_EDGES       # 126976

XV_COLS = BLK_EDGES            # 4096 fp16 cols of [v_i^T; v_j^T]
E_COLS = BLK_EDGES // 4        # 1024 fp16 cols of folded e^T
IN_COLS = XV_COLS + E_COLS     # 5120

N_WARM = 18                    # warmup matmuls bridging until block-0 data

F32 = mybir.dt.float32
F16 = mybir.dt.float16

# test.py hooks
_TRACE = False
LAST_RESULT = None

_PROGRAM_CACHE = {}


def _build_program():
    nc = bacc.Bacc(
        "TRN2",
        target_bir_lowering=False,
        debug=False,
        num_devices=N_CORES,
    )

    xin = nc.declare_dram_parameter(
        "xin", [N_BLK, 128, IN_COLS], F16, isOutput=False
    )
    # wf16 packs [W1v | We*4 | W2*2] -> [128, 192] fp16, one DMA
    wf16 = nc.declare_dram_parameter("wf16", [128, 3 * HID], F16, isOutput=False)
    bf32 = nc.declare_dram_parameter("bf32", [128, 2], F32, isOutput=False)
    out = nc.declare_dram_parameter(
        "out", [N_BLK, 128, 4 * NHALF], F16, isOutput=True
    )

    with tile.TileContext(nc) as tc:
        with (
            tc.tile_pool(name="consts", bufs=1) as cpool,
            tc.tile_pool(name="xi", bufs=7) as xi_pool,
            tc.tile_pool(name="hb", bufs=4) as h_pool,
            tc.tile_pool(name="ob", bufs=4) as ob_pool,
            tc.tile_pool(name="ph", bufs=4, space="PSUM") as ph_pool,
            tc.tile_pool(name="po", bufs=4, space="PSUM") as po_pool,
        ):
            w_t = cpool.tile([128, 3 * HID], F16)
            b_t = cpool.tile([128, 2], F32)
            warm_t = cpool.tile([128, NHALF], F16)

            nc.sync.dma_start(w_t[:], wf16[:])
            nc.sync.dma_start(b_t[:], bf32[:])
            w1a = w_t[:, 0:HID]            # [128,64]: rows 0-63 W1vi, 64-127 W1vj
            wer = w_t[:, HID : 2 * HID]    # [128,64]: We tiled on 4 row groups
            w2r = w_t[:, 2 * HID : 3 * HID]  # [128,64]: W2 tiled on 2 row halves
            b1c = b_t[:, 0:1]
            b2c = b_t[:, 1:2]

            nc.vector.memset(warm_t[:], 0.0)
            warm_ps = ph_pool.tile([128, NHALF], F32, tag="ph_t", name="warm_ps")
            for _ in range(N_WARM):
                nc.tensor.matmul(
                    warm_ps[:, :], warm_t[:, 0:128], warm_t[:, :],
                    start=True, stop=True,
                )

            for blk in range(N_BLK):
                xi_t = xi_pool.tile([128, IN_COLS], F16)
                nc.sync.dma_start(xi_t[:], xin[blk])
                ob_t = ob_pool.tile([128, 4 * NHALF], F16)

                # dependency-free filler matmuls: keep the PE the pacing
                # engine (~5.5us/block vs ~5.1us DMA). Any >=1us PE idle
                # gap re-throttles the clock to 1.2 GHz permanently
                # (measured), costing far more than the filler. The last
                # blocks need no pacing protection.
                for _ in range(3 if blk < N_BLK - 2 else 0):
                    nc.tensor.matmul(
                        warm_ps[:, :], warm_t[:, 0:128], warm_t[:, :],
                        start=True, stop=True,
                    )

                # phase 1: all pairs' L1 steps + vector relu. Keeping the
                # L2 matmuls in a separate phase stops the Tile scheduler
                # from interleaving a ready L2 step between the two halves
                # of a later pair's xv/e step, which breaks their
                # co-execution (measured: 2 split pairs = +0.66us/block).
                h_ts = []
                for pair in range(Q_PER_BLK // 2):
                    # tiles A = pair (col group 0), B = pair + 4 (col group 1)
                    ph_t = ph_pool.tile([128, NHALF], F32, name="ph_t")
                    # step 1: xv matmuls on both column groups
                    for par in range(2):
                        c0 = 64 * par
                        q = pair + 4 * par
                        nc.tensor.matmul(
                            ph_t[c0 : c0 + 64, :],
                            w1a[:, :],
                            xi_t[:, bass.ts(q, NHALF)],
                            start=True, stop=False, tile_position=(0, c0),
                        )
                    # step 2: e matmuls, diagonal row groups (chunks differ
                    # by 2 between tile A and tile B)
                    for par in range(2):
                        c0 = 64 * par
                        q = pair + 4 * par
                        r0 = 32 * (q // 2)
                        ec0 = XV_COLS + (q % 2) * NHALF
                        nc.tensor.matmul(
                            ph_t[c0 : c0 + 64, :],
                            wer[r0 : r0 + 32, :],
                            xi_t[r0 : r0 + 32, ec0 : ec0 + NHALF],
                            start=False, stop=True, tile_position=(r0, c0),
                        )
                    h_t = h_pool.tile([128, NHALF], F16, name="h_t")
                    nc.vector.tensor_scalar(
                        h_t[:],
                        ph_t[:],
                        b1c,
                        0.0,
                        mybir.AluOpType.add,
                        mybir.AluOpType.max,
                    )
                    h_ts.append(h_t)
                # phase 2: all pairs' L2 steps + output copy
                for pair in range(Q_PER_BLK // 2):
                    po_t = po_pool.tile([128, NHALF], F32, name="po_t")
                    for par in range(2):
                        c0 = 64 * par
                        nc.tensor.matmul(
                            po_t[c0 : c0 + 64, :],
                            w2r[c0 : c0 + 64, :],
                            h_ts[pair][c0 : c0 + 64, :],
                            start=True, stop=True, tile_position=(c0, c0),
                        )
                    nc.scalar.activation(
                        ob_t[:, bass.ts(pair, NHALF)],
                        po_t[:],
                        mybir.ActivationFunctionType.Identity,
                        bias=b2c,
                    )
                nc.sync.dma_start(out[blk], ob_t[:])

    nc.compile()
    return nc


def _get_program():
    if "prog" not in _PROGRAM_CACHE:
        _PROGRAM_CACHE["prog"] = _build_program()
    return _PROGRAM_CACHE["prog"]


def _pad_rows(a, n):
    if a.shape[0] == n:
        return a
    pad = np.zeros((n - a.shape[0],) + a.shape[1:], dtype=a.dtype)
    return np.concatenate([a, pad], axis=0)


def _host_pack(v_i, v_j, e_ij, W1, b1, W2, b2):
    """Build per-core input maps in the device layouts."""
    v_i = np.ascontiguousarray(v_i, dtype=np.float32)
    v_j = np.ascontiguousarray(v_j, dtype=np.float32)
    e_ij = np.ascontiguousarray(e_ij, dtype=np.float32)

    W1v = np.asarray(W1[:128], dtype=np.float16)          # [128, 64]
    We = np.asarray(W1[128:160], dtype=np.float16)        # [32, 64]
    W2h = np.asarray(W2, dtype=np.float16)                # [64, 64]

    wf16 = np.concatenate(
        [W1v, np.tile(We, (4, 1)), np.tile(W2h, (2, 1))], axis=1
    )  # [128, 192]
    bf32 = np.stack(
        [np.tile(b1, 2), np.tile(b2, 2)], axis=1
    ).astype(np.float32)  # [128, 2]

    weights = {
        "wf16": np.ascontiguousarray(wf16),
        "bf32": np.ascontiguousarray(bf32),
    }

    in_maps = []
    for c in range(N_CORES):
        sl = slice(c * EPC, (c + 1) * EPC)
        vi = _pad_rows(v_i[sl], EPAD)    # [EPAD, 64]
        vj = _pad_rows(v_j[sl], EPAD)
        ec = _pad_rows(e_ij[sl], EPAD)   # [EPAD, 32]

        A = np.concatenate([vi.T, vj.T], axis=0).astype(np.float16)
        # [128, EPAD] -> per block [N_BLK, 128, 4096]
        Ax = A.reshape(128, N_BLK, XV_COLS).transpose(1, 0, 2)

        # e^T [32, EPAD] folded: chunk c (1024 edges) -> partitions 32c:32c+32
        ET = ec.T.astype(np.float16)                      # [32, EPAD]
        Ef = ET.reshape(32, N_BLK, 4, E_COLS).transpose(1, 2, 0, 3)
        Ef = Ef.reshape(N_BLK, 128, E_COLS)               # [N_BLK, 128, 1024]

        xin = np.concatenate([Ax, Ef], axis=2)            # [N_BLK, 128, 5120]
        in_maps.append({"xin": np.ascontiguousarray(xin), **weights})
    return in_maps


def _host_unpack(results):
    """results: list of per-core dicts with 'out' [N_BLK, 128, 2048] f16."""
    outs = []
    for c in range(N_CORES):
        o = np.asarray(results[c]["out"])
        # out[b, 64*par + ch, 512*pair + n] =
        #   OUT[b*4096 + par*2048 + pair*512 + n, ch]
        r = o.reshape(N_BLK, 2, OUT_C, 4, NHALF)   # [b, par, ch, pair, n]
        r = r.transpose(0, 1, 3, 4, 2)             # [b, par, pair, n, ch]
        outs.append(
            np.ascontiguousarray(r).reshape(EPAD, OUT_C)[:EPC]
        )
    return np.concatenate(outs, axis=0).astype(np.float32)


def kernel(v_i, v_j, e_ij, W1, b1, W2, b2):
    global LAST_RESULT
    nc = _get_program()
    in_maps = _host_pack(v_i, v_j, e_ij, W1, b1, W2, b2)
    res = run_bass_kernel_spmd(
        nc, in_maps, core_ids=list(range(N_CORES)), trace=_TRACE
    )
    LAST_RESULT = res
    return _host_unpack(res.results)


# revision 21
# speedup vs baseline: 1.0862x; 1.0862x over previous
"""Trainium2 Bass kernel for the GNN message-update MLP:

    out = relu(concat([v_i, v_j, e_ij], -1) @ W1 + b1) @ W2 + b2

Strategy (memory-bound, E = 1M edges, data-parallel across 8 cores):
  - Shard edges across the 8 NeuronCores (125000 each, padded to 126976).
  - Everything ships in fp16 (inputs AND outputs): the harness gate is
    rel_err < 2e-2 and fp16 end-to-end measures ~6e-4, so fp32
    split-precision would waste 2x the HBM traffic. Per-core traffic is
    ~57MB -> ~160us at the 358 GB/s HBM/core roofline.
  - One fused input DMA per 4096-edge block: [128, 5120] fp16 where
    cols 0:4096 hold [v_i^T; v_j^T] (128 partitions) and cols 4096:5120
    hold e^T folded from [32, 4096] to [128, 1024] (chunk c of 1024
    edges lives on partitions 32c:32c+32) so the DMA uses all 128
    partitions / all 16 SDMA engines.
  - Matmuls co-execute only when their PE quadrant sets are disjoint
    AND they write disjoint PSUM partition ranges (two concurrent
    streams into the same PSUM region race on has_written - hangs the
    device, measured). So tiles are paired (q, q+4) across the two PE
    column groups and issued as steps of two disjoint-quadrant matmuls:
      step1: xv_A@(0,cg0) , xv_B@(0,cg1)       K=128 each
      step2: e_A@(32c,cg0), e_B@(32c+64,cg1)   K=32, diagonal row grps
             (pairing q with q+4 puts their e-chunks on different row
             halves; e overlaps xv's quadrant so same-PSUM accumulation
             is hardware-serialized - safe)
      VectorE: h = fp16(relu(psum + b1))       one [128,512] op
      step3: L2_A@(0,0) , L2_B@(64,64)         K=64, diagonal
      ScalarE: out = fp16(psum + b2)           one [128,512] op
    ~3 x 512-cycle steps per 1024 edges -> ~80-106us/core PE time,
    under the ~160us DMA floor.
  - Startup ordered to keep the PE HAM clock gate warm: weights fused
    into 2 small DMAs ahead of block-0's input DMA, and the warmup
    matmul run sized to bridge until block-0 data lands. A PE idle gap
    >= ~1us re-throttles the PE clock to 1.2 GHz, usually for the rest
    of the kernel (measured) - hence the warmup, the 7-deep input
    prefetch, and the per-block filler matmuls that keep the PE the
    pacing engine.
  - Output [128, 2048] fp16 per block, unfolded + cast to f32 on host.

Measured: 190.5us HW exec (8 cores), rel err 5.7e-4 vs the fp32
reference (gate 2e-2). Baseline this session started from: 482.9us.
"""

import numpy as np

import concourse.bacc as bacc
import concourse.bass as bass
import concourse.mybir as mybir
import concourse.tile as tile
from concourse.bass_utils import run_bass_kernel_spmd

# ---- problem constants (hardcoded per harness contract) ----
E_TOTAL = 1_000_000
N_CORES = 8
IN_C = 64
IN_E = 32
HID = 64
OUT_C = 64

NHALF = 512                    # edges per matmul (moving free dim, 1 psum bank)
Q_PER_BLK = 8                  # 512-edge tiles per block
BLK_EDGES = NHALF * Q_PER_BLK  # 4096
EPC = E_TOTAL // N_CORES       # 125000 edges per core
N_BLK = -(-EPC // BLK_EDGES)   # 31
EPAD = N_BLK * BLK_EDGES       # 126976

XV_COLS = BLK_EDGES            # 4096 fp16 cols of [v_i^T; v_j^T]
E_COLS = BLK_EDGES // 4        # 1024 fp16 cols of folded e^T
IN_COLS = XV_COLS + E_COLS     # 5120

N_WARM = 18                    # warmup matmuls bridging until block-0 data

F32 = mybir.dt.float32
F16 = mybir.dt.float16

# test.py hooks
_TRACE = False
LAST_RESULT = None

_PROGRAM_CACHE = {}


def _build_program():
    nc = bacc.Bacc(
        "TRN2",
        target_bir_lowering=False,
        debug=False,
        num_devices=N_CORES,
    )

    xin = nc.declare_dram_parameter(
        "xin", [N_BLK, 128, IN_COLS], F16, isOutput=False
    )
    # wf16 packs [W1v | We*4 | W2*2] -> [128, 192] fp16, one DMA
    wf16 = nc.declare_dram_parameter("wf16", [128, 3 * HID], F16, isOutput=False)
    bf32 = nc.declare_dram_parameter("bf32", [128, 2], F32, isOutput=False)
    out = nc.declare_dram_parameter(
        "out", [N_BLK, 128, 4 * NHALF], F16, isOutput=True
    )

    with tile.TileContext(nc) as tc:
        with (
            tc.tile_pool(name="consts", bufs=1) as cpool,
            tc.tile_pool(name="xi", bufs=7) as xi_pool,
            tc.tile_pool(name="hb", bufs=4) as h_pool,
            tc.tile_pool(name="ob", bufs=4) as ob_pool,
            tc.tile_pool(name="ph", bufs=4, space="PSUM") as ph_pool,
            tc.tile_pool(name="po", bufs=4, space="PSUM") as po_pool,
        ):
            w_t = cpool.tile([128, 3 * HID], F16)
            b_t = cpool.tile([128, 2], F32)
            warm_t = cpool.tile([128, NHALF], F16)

            nc.sync.dma_start(w_t[:], wf16[:])
            nc.sync.dma_start(b_t[:], bf32[:])
            w1a = w_t[:, 0:HID]            # [128,64]: rows 0-63 W1vi, 64-127 W1vj
            wer = w_t[:, HID : 2 * HID]    # [128,64]: We tiled on 4 row groups
            w2r = w_t[:, 2 * HID : 3 * HID]  # [128,64]: W2 tiled on 2 row halves
            b1c = b_t[:, 0:1]
            b2c = b_t[:, 1:2]

            nc.vector.memset(warm_t[:], 0.0)
            warm_ps = ph_pool.tile([128, NHALF], F32, tag="ph_t", name="warm_ps")
            for _ in range(N_WARM):
                nc.tensor.matmul(
                    warm_ps[:, :], warm_t[:, 0:128], warm_t[:, :],
                    start=True, stop=True,
                )

            for blk in range(N_BLK):
                xi_t = xi_pool.tile([128, IN_COLS], F16)
                nc.sync.dma_start(xi_t[:], xin[blk])
                ob_t = ob_pool.tile([128, 4 * NHALF], F16)

                # dependency-free filler matmuls: keep the PE the pacing
                # engine (~5.5us/block vs ~5.1us DMA). Any >=1us PE idle
                # gap re-throttles the clock to 1.2 GHz permanently
                # (measured), costing far more than the filler. The last
                # blocks need no pacing protection.
                # the DMA lead built during ramp erodes ~0.2us/block (the
                # slowest SDMA engines pace slightly above the mean), so
                # late blocks get one extra filler to keep the PE from
                # hitting the >=1us-gap throttle cliff near the end.
                n_fill = 3 if blk < 18 else (4 if blk < N_BLK - 2 else 0)
                for _ in range(n_fill):
                    nc.tensor.matmul(
                        warm_ps[:, :], warm_t[:, 0:128], warm_t[:, :],
                        start=True, stop=True,
                    )

                # phase 1: all pairs' L1 steps + vector relu. Keeping the
                # L2 matmuls in a separate phase stops the Tile scheduler
                # from interleaving a ready L2 step between the two halves
                # of a later pair's xv/e step, which breaks their
                # co-execution (measured: 2 split pairs = +0.66us/block).
                h_ts = []
                for pair in range(Q_PER_BLK // 2):
                    # tiles A = pair (col group 0), B = pair + 4 (col group 1)
                    ph_t = ph_pool.tile([128, NHALF], F32, name="ph_t")
                    # step 1: xv matmuls on both column groups
                    for par in range(2):
                        c0 = 64 * par
                        q = pair + 4 * par
                        nc.tensor.matmul(
                            ph_t[c0 : c0 + 64, :],
                            w1a[:, :],
                            xi_t[:, bass.ts(q, NHALF)],
                            start=True, stop=False, tile_position=(0, c0),
                        )
                    # step 2: e matmuls, diagonal row groups (chunks differ
                    # by 2 between tile A and tile B)
                    for par in range(2):
                        c0 = 64 * par
                        q = pair + 4 * par
                        r0 = 32 * (q // 2)
                        ec0 = XV_COLS + (q % 2) * NHALF
                        nc.tensor.matmul(
                            ph_t[c0 : c0 + 64, :],
                            wer[r0 : r0 + 32, :],
                            xi_t[r0 : r0 + 32, ec0 : ec0 + NHALF],
                            start=False, stop=True, tile_position=(r0, c0),
                        )
                    h_t = h_pool.tile([128, NHALF], F16, name="h_t")
                    nc.vector.tensor_scalar(
                        h_t[:],
                        ph_t[:],
                        b1c,
                        0.0,
                        mybir.AluOpType.add,
                        mybir.AluOpType.max,
                    )
                    h_ts.append(h_t)
                # phase 2: all pairs' L2 steps + output copy
                for pair in range(Q_PER_BLK // 2):
                    po_t = po_pool.tile([128, NHALF], F32, name="po_t")
                    for par in range(2):
                        c0 = 64 * par
                        nc.tensor.matmul(
                            po_t[c0 : c0 + 64, :],
                            w2r[c0 : c0 + 64, :],
                            h_ts[pair][c0 : c0 + 64, :],
                            start=True, stop=True, tile_position=(c0, c0),
                        )
                    nc.scalar.activation(
                        ob_t[:, bass.ts(pair, NHALF)],
                        po_t[:],
                        mybir.ActivationFunctionType.Identity,
                        bias=b2c,
                    )
                nc.sync.dma_start(out[blk], ob_t[:])

    nc.compile()
    return nc


def _get_program():
    if "prog" not in _PROGRAM_CACHE:
        _PROGRAM_CACHE["prog"] = _build_program()
    return _PROGRAM_CACHE["prog"]


def _pad_rows(a, n):
    if a.shape[0] == n:
        return a
    pad = np.zeros((n - a.shape[0],) + a.shape[1:], dtype=a.dtype)
    return np.concatenate([a, pad], axis=0)


def _host_pack(v_i, v_j, e_ij, W1, b1, W2, b2):
    """Build per-core input maps in the device layouts."""
    v_i = np.ascontiguousarray(v_i, dtype=np.float32)
    v_j = np.ascontiguousarray(v_j, dtype=np.float32)
    e_ij = np.ascontiguousarray(e_ij, dtype=np.float32)

    W1v = np.asarray(W1[:128], dtype=np.float16)          # [128, 64]
    We = np.asarray(W1[128:160], dtype=np.float16)        # [32, 64]
    W2h = np.asarray(W2, dtype=np.float16)                # [64, 64]

    wf16 = np.concatenate(
        [W1v, np.tile(We, (4, 1)), np.tile(W2h, (2, 1))], axis=1
    )  # [128, 192]
    bf32 = np.stack(
        [np.tile(b1, 2), np.tile(b2, 2)], axis=1
    ).astype(np.float32)  # [128, 2]

    weights = {
        "wf16": np.ascontiguousarray(wf16),
        "bf32": np.ascontiguousarray(bf32),
    }

    in_maps = []
    for c in range(N_CORES):
        sl = slice(c * EPC, (c + 1) * EPC)
        vi = _pad_rows(v_i[sl], EPAD)    # [EPAD, 64]
        vj = _pad_rows(v_j[sl], EPAD)
        ec = _pad_rows(e_ij[sl], EPAD)   # [EPAD, 32]

        A = np.concatenate([vi.T, vj.T], axis=0).astype(np.float16)
        # [128, EPAD] -> per block [N_BLK, 128, 4096]
        Ax = A.reshape(128, N_BLK, XV_COLS).transpose(1, 0, 2)

        # e^T [32, EPAD] folded: chunk c (1024 edges) -> partitions 32c:32c+32
        ET = ec.T.astype(np.float16)                      # [32, EPAD]
        Ef = ET.reshape(32, N_BLK, 4, E_COLS).transpose(1, 2, 0, 3)
        Ef = Ef.reshape(N_BLK, 128, E_COLS)               # [N_BLK, 128, 1024]

        xin = np.concatenate([Ax, Ef], axis=2)            # [N_BLK, 128, 5120]
        in_maps.append({"xin": np.ascontiguousarray(xin), **weights})
    return in_maps


def _host_unpack(results):
    """results: list of per-core dicts with 'out' [N_BLK, 128, 2048] f16."""
    outs = []
    for c in range(N_CORES):
        o = np.asarray(results[c]["out"])
        # out[b, 64*par + ch, 512*pair + n] =
        #   OUT[b*4096 + par*2048 + pair*512 + n, ch]
        r = o.reshape(N_BLK, 2, OUT_C, 4, NHALF)   # [b, par, ch, pair, n]
        r = r.transpose(0, 1, 3, 4, 2)             # [b, par, pair, n, ch]
        outs.append(
            np.ascontiguousarray(r).reshape(EPAD, OUT_C)[:EPC]
        )
    return np.concatenate(outs, axis=0).astype(np.float32)


def kernel(v_i, v_j, e_ij, W1, b1, W2, b2):
    global LAST_RESULT
    nc = _get_program()
    in_maps = _host_pack(v_i, v_j, e_ij, W1, b1, W2, b2)
    res = run_bass_kernel_spmd(
        nc, in_maps, core_ids=list(range(N_CORES)), trace=_TRACE
    )
    LAST_RESULT = res
    return _host_unpack(res.results)


# revision 22
# speedup vs baseline: 1.1700x; 1.0772x over previous
"""Trainium2 Bass kernel for the GNN message-update MLP:

    out = relu(concat([v_i, v_j, e_ij], -1) @ W1 + b1) @ W2 + b2

Strategy (memory-bound, E = 1M edges, data-parallel across 8 cores):
  - Shard edges across the 8 NeuronCores (125000 each, padded to 126976).
  - Everything ships in fp16 (inputs AND outputs): the harness gate is
    rel_err < 2e-2 and fp16 end-to-end measures ~6e-4, so fp32
    split-precision would waste 2x the HBM traffic. Per-core traffic is
    ~57MB -> ~160us at the 358 GB/s HBM/core roofline.
  - One fused input DMA per 4096-edge block: [128, 5120] fp16 where
    cols 0:4096 hold [v_i^T; v_j^T] (128 partitions) and cols 4096:5120
    hold e^T folded from [32, 4096] to [128, 1024] (chunk c of 1024
    edges lives on partitions 32c:32c+32) so the DMA uses all 128
    partitions / all 16 SDMA engines.
  - Matmuls co-execute only when their PE quadrant sets are disjoint
    AND they write disjoint PSUM partition ranges (two concurrent
    streams into the same PSUM region race on has_written - hangs the
    device, measured). So tiles are paired (q, q+4) across the two PE
    column groups and issued as steps of two disjoint-quadrant matmuls:
      step1: xv_A@(0,cg0) , xv_B@(0,cg1)       K=128 each
      step2: e_A@(32c,cg0), e_B@(32c+64,cg1)   K=32, diagonal row grps
             (pairing q with q+4 puts their e-chunks on different row
             halves; e overlaps xv's quadrant so same-PSUM accumulation
             is hardware-serialized - safe)
      VectorE: h = fp16(relu(psum + b1))       one [128,512] op
      step3: L2_A@(0,0) , L2_B@(64,64)         K=64, diagonal
      ScalarE: out = fp16(psum + b2)           one [128,512] op
    ~3 x 512-cycle steps per 1024 edges -> ~80-106us/core PE time,
    under the ~160us DMA floor.
  - Startup ordered to keep the PE HAM clock gate warm: weights fused
    into 2 small DMAs ahead of block-0's input DMA, and the warmup
    matmul run sized to bridge until block-0 data lands. A PE idle gap
    >= ~1us re-throttles the PE clock to 1.2 GHz, usually for the rest
    of the kernel (measured) - hence the warmup, the 7-deep input
    prefetch, and the per-block filler matmuls that keep the PE the
    pacing engine.
  - Output [128, 2048] fp16 per block, unfolded + cast to f32 on host.

Measured: 173-189us HW exec (8 cores; varies with chip power state and
HBM contention), rel err 5.7e-4 vs the fp32 reference (gate 2e-2).
Baseline this session started from: 482.9us.
"""

import numpy as np

import concourse.bacc as bacc
import concourse.bass as bass
import concourse.mybir as mybir
import concourse.tile as tile
from concourse.bass_utils import run_bass_kernel_spmd

# ---- problem constants (hardcoded per harness contract) ----
E_TOTAL = 1_000_000
N_CORES = 8
IN_C = 64
IN_E = 32
HID = 64
OUT_C = 64

NHALF = 512                    # edges per matmul (moving free dim, 1 psum bank)
Q_PER_BLK = 8                  # 512-edge tiles per block
BLK_EDGES = NHALF * Q_PER_BLK  # 4096
EPC = E_TOTAL // N_CORES       # 125000 edges per core
N_BLK = -(-EPC // BLK_EDGES)   # 31
EPAD = N_BLK * BLK_EDGES       # 126976

XV_COLS = BLK_EDGES            # 4096 fp16 cols of [v_i^T; v_j^T]
E_COLS = BLK_EDGES // 4        # 1024 fp16 cols of folded e^T
IN_COLS = XV_COLS + E_COLS     # 5120

N_WARM = 18                    # warmup matmuls bridging until block-0 data

F32 = mybir.dt.float32
F16 = mybir.dt.float16

# test.py hooks
_TRACE = False
LAST_RESULT = None

_PROGRAM_CACHE = {}


def _build_program():
    nc = bacc.Bacc(
        "TRN2",
        target_bir_lowering=False,
        debug=False,
        num_devices=N_CORES,
    )

    xin = nc.declare_dram_parameter(
        "xin", [N_BLK, 128, IN_COLS], F16, isOutput=False
    )
    # wf16 packs [W1v | We*4 | W2*2] -> [128, 192] fp16, one DMA
    wf16 = nc.declare_dram_parameter("wf16", [128, 3 * HID], F16, isOutput=False)
    bf32 = nc.declare_dram_parameter("bf32", [128, 2], F32, isOutput=False)
    out = nc.declare_dram_parameter(
        "out", [N_BLK, 128, 4 * NHALF], F16, isOutput=True
    )

    with tile.TileContext(nc) as tc:
        with (
            tc.tile_pool(name="consts", bufs=1) as cpool,
            tc.tile_pool(name="xi", bufs=7) as xi_pool,
            tc.tile_pool(name="hb", bufs=4) as h_pool,
            tc.tile_pool(name="ob", bufs=4) as ob_pool,
            tc.tile_pool(name="ph", bufs=4, space="PSUM") as ph_pool,
            tc.tile_pool(name="po", bufs=4, space="PSUM") as po_pool,
        ):
            w_t = cpool.tile([128, 3 * HID], F16)
            b_t = cpool.tile([128, 2], F32)
            warm_t = cpool.tile([128, NHALF], F16)

            nc.sync.dma_start(w_t[:], wf16[:])
            nc.sync.dma_start(b_t[:], bf32[:])
            w1a = w_t[:, 0:HID]            # [128,64]: rows 0-63 W1vi, 64-127 W1vj
            wer = w_t[:, HID : 2 * HID]    # [128,64]: We tiled on 4 row groups
            w2r = w_t[:, 2 * HID : 3 * HID]  # [128,64]: W2 tiled on 2 row halves
            b1c = b_t[:, 0:1]
            b2c = b_t[:, 1:2]

            nc.vector.memset(warm_t[:], 0.0)
            warm_ps = ph_pool.tile([128, NHALF], F32, tag="ph_t", name="warm_ps")
            for _ in range(N_WARM):
                nc.tensor.matmul(
                    warm_ps[:, :], warm_t[:, 0:128], warm_t[:, :],
                    start=True, stop=True,
                )

            for blk in range(N_BLK):
                xi_t = xi_pool.tile([128, IN_COLS], F16)
                nc.sync.dma_start(xi_t[:], xin[blk])
                ob_t = ob_pool.tile([128, 4 * NHALF], F16)

                # dependency-free filler matmuls: keep the PE the pacing
                # engine (~5.5us/block vs ~5.1us DMA). Any >=1us PE idle
                # gap re-throttles the clock to 1.2 GHz permanently
                # (measured), costing far more than the filler. The last
                # blocks need no pacing protection.
                # the DMA lead built during ramp erodes ~0.2us/block (the
                # slowest SDMA engines pace slightly above the mean), so
                # late blocks get one extra filler to keep the PE from
                # hitting the >=1us-gap throttle cliff near the end.
                n_fill = 3 if blk < 18 else (4 if blk < N_BLK - 2 else 0)
                for _ in range(n_fill):
                    nc.tensor.matmul(
                        warm_ps[:, :], warm_t[:, 0:128], warm_t[:, :],
                        start=True, stop=True,
                    )

                # phase 1: all pairs' L1 steps + vector relu. Keeping the
                # L2 matmuls in a separate phase stops the Tile scheduler
                # from interleaving a ready L2 step between the two halves
                # of a later pair's xv/e step, which breaks their
                # co-execution (measured: 2 split pairs = +0.66us/block).
                h_ts = []
                for pair in range(Q_PER_BLK // 2):
                    # tiles A = pair (col group 0), B = pair + 4 (col group 1)
                    ph_t = ph_pool.tile([128, NHALF], F32, name="ph_t")
                    # step 1: xv matmuls on both column groups
                    for par in range(2):
                        c0 = 64 * par
                        q = pair + 4 * par
                        nc.tensor.matmul(
                            ph_t[c0 : c0 + 64, :],
                            w1a[:, :],
                            xi_t[:, bass.ts(q, NHALF)],
                            start=True, stop=False, tile_position=(0, c0),
                        )
                    # step 2: e matmuls, diagonal row groups (chunks differ
                    # by 2 between tile A and tile B)
                    for par in range(2):
                        c0 = 64 * par
                        q = pair + 4 * par
                        r0 = 32 * (q // 2)
                        ec0 = XV_COLS + (q % 2) * NHALF
                        nc.tensor.matmul(
                            ph_t[c0 : c0 + 64, :],
                            wer[r0 : r0 + 32, :],
                            xi_t[r0 : r0 + 32, ec0 : ec0 + NHALF],
                            start=False, stop=True, tile_position=(r0, c0),
                        )
                    h_t = h_pool.tile([128, NHALF], F16, name="h_t")
                    nc.vector.tensor_scalar(
                        h_t[:],
                        ph_t[:],
                        b1c,
                        0.0,
                        mybir.AluOpType.add,
                        mybir.AluOpType.max,
                    )
                    h_ts.append(h_t)
                # phase 2: all pairs' L2 steps + output copy
                for pair in range(Q_PER_BLK // 2):
                    po_t = po_pool.tile([128, NHALF], F32, name="po_t")
                    for par in range(2):
                        c0 = 64 * par
                        nc.tensor.matmul(
                            po_t[c0 : c0 + 64, :],
                            w2r[c0 : c0 + 64, :],
                            h_ts[pair][c0 : c0 + 64, :],
                            start=True, stop=True, tile_position=(c0, c0),
                        )
                    nc.scalar.activation(
                        ob_t[:, bass.ts(pair, NHALF)],
                        po_t[:],
                        mybir.ActivationFunctionType.Identity,
                        bias=b2c,
                    )
                nc.sync.dma_start(out[blk], ob_t[:])

    nc.compile()
    return nc


def _get_program():
    if "prog" not in _PROGRAM_CACHE:
        _PROGRAM_CACHE["prog"] = _build_program()
    return _PROGRAM_CACHE["prog"]


def _pad_rows(a, n):
    if a.shape[0] == n:
        return a
    pad = np.zeros((n - a.shape[0],) + a.shape[1:], dtype=a.dtype)
    return np.concatenate([a, pad], axis=0)


def _host_pack(v_i, v_j, e_ij, W1, b1, W2, b2):
    """Build per-core input maps in the device layouts."""
    v_i = np.ascontiguousarray(v_i, dtype=np.float32)
    v_j = np.ascontiguousarray(v_j, dtype=np.float32)
    e_ij = np.ascontiguousarray(e_ij, dtype=np.float32)

    W1v = np.asarray(W1[:128], dtype=np.float16)          # [128, 64]
    We = np.asarray(W1[128:160], dtype=np.float16)        # [32, 64]
    W2h = np.asarray(W2, dtype=np.float16)                # [64, 64]

    wf16 = np.concatenate(
        [W1v, np.tile(We, (4, 1)), np.tile(W2h, (2, 1))], axis=1
    )  # [128, 192]
    bf32 = np.stack(
        [np.tile(b1, 2), np.tile(b2, 2)], axis=1
    ).astype(np.float32)  # [128, 2]

    weights = {
        "wf16": np.ascontiguousarray(wf16),
        "bf32": np.ascontiguousarray(bf32),
    }

    in_maps = []
    for c in range(N_CORES):
        sl = slice(c * EPC, (c + 1) * EPC)
        vi = _pad_rows(v_i[sl], EPAD)    # [EPAD, 64]
        vj = _pad_rows(v_j[sl], EPAD)
        ec = _pad_rows(e_ij[sl], EPAD)   # [EPAD, 32]

        A = np.concatenate([vi.T, vj.T], axis=0).astype(np.float16)
        # [128, EPAD] -> per block [N_BLK, 128, 4096]
        Ax = A.reshape(128, N_BLK, XV_COLS).transpose(1, 0, 2)

        # e^T [32, EPAD] folded: chunk c (1024 edges) -> partitions 32c:32c+32
        ET = ec.T.astype(np.float16)                      # [32, EPAD]
        Ef = ET.reshape(32, N_BLK, 4, E_COLS).transpose(1, 2, 0, 3)
        Ef = Ef.reshape(N_BLK, 128, E_COLS)               # [N_BLK, 128, 1024]

        xin = np.concatenate([Ax, Ef], axis=2)            # [N_BLK, 128, 5120]
        in_maps.append({"xin": np.ascontiguousarray(xin), **weights})
    return in_maps


def _host_unpack(results):
    """results: list of per-core dicts with 'out' [N_BLK, 128, 2048] f16."""
    outs = []
    for c in range(N_CORES):
        o = np.asarray(results[c]["out"])
        # out[b, 64*par + ch, 512*pair + n] =
        #   OUT[b*4096 + par*2048 + pair*512 + n, ch]
        r = o.reshape(N_BLK, 2, OUT_C, 4, NHALF)   # [b, par, ch, pair, n]
        r = r.transpose(0, 1, 3, 4, 2)             # [b, par, pair, n, ch]
        outs.append(
            np.ascontiguousarray(r).reshape(EPAD, OUT_C)[:EPC]
        )
    return np.concatenate(outs, axis=0).astype(np.float32)


def kernel(v_i, v_j, e_ij, W1, b1, W2, b2):
    global LAST_RESULT
    nc = _get_program()
    in_maps = _host_pack(v_i, v_j, e_ij, W1, b1, W2, b2)
    res = run_bass_kernel_spmd(
        nc, in_maps, core_ids=list(range(N_CORES)), trace=_TRACE
    )
    LAST_RESULT = res
    return _host_unpack(res.results)
